# revision 19
# baseline (speedup 1.0000x reference)
"""Trainium2 Bass kernel for nn_BaseGNNModel (2-layer GCN + image-query matmul).

Math (reference):
    norm = dinv[src] * w * dinv[dst],  dinv = rsqrt(segment_sum(w, dst))
    x1 = leaky_relu(segsum(norm * (NF @ W1)[src], dst) + b1, 0.2)
    x2 = segsum(norm * (x1 @ W2)[src], dst) + b2
    out = img @ x2.T                                  # [64, 20000]

Algebraic restructure (exact up to fp reassociation):
    aggF = segsum(norm * NF[src], dst)                 # matmul commutes with segsum
    x1T  = leaky_relu(W1.T @ aggF + b1)                # [HID, N]
    PT   = W2 @ imgT                                   # [HID, B]
    QTT  = PT.T @ x1T                                  # [B, N]
    out[:, n] = segsum(norm * QTT.T[src], dst).T + img @ b2

Sharding: nodes (and their incoming edges) are range-sharded across the 8
cores; every segment-sum output is fully core-local.  Three SPMD launches:
  L1: per-core degree -> dinv  [NB, nblk] per core
  L2: layer-1 aggregation (gather + one-hot selection matmuls), x1T, PT, QTT
  L3: final aggregation over gathered QT rows -> output shard [64, N/8]
Host work between launches is layout only (reshape/concat of dinv and QTT
shards, np indexing of the dinv table by edge endpoints, dtype casts); all
arithmetic runs on the NeuronCores.  Segment sums are PE matmuls against
one-hot selection matrices built on-device with iota + fused is_equal*norm,
so duplicate destinations accumulate exactly in fp32 PSUM.
"""

import numpy as np
import ml_dtypes

from concourse import bacc, bass, mybir
from concourse.bass_utils import run_bass_kernel_spmd
from concourse.masks import make_identity
from concourse.tile import TileContext

P = 128
NB = 125            # nodes per block (psum free dim)
F_TEXT = 300
FPAD = 384          # bf16 node-feature row: 384*2B = 768B (256B multiple)
NEG = 0.2

CFG_FULL = dict(B=64, N=20000, E=160000, HID=1024, OUT=1664, CORES=8)

TRACE = False
LAST_EXEC_NS = {}

_BUILD_CACHE = {}


# ----------------------------------------------------------------- host prep

def _prep_edges(edge_src, edge_dst, edge_weight, cfg):
    """Group edges by (core, block) of their dst; pad each block's edge list
    to T_b*128 where T_b is the max tile count for block index b across
    cores (SPMD needs identical program structure on every core)."""
    ncores = cfg["CORES"]
    npc = cfg["N"] // ncores
    nblk = npc // NB
    assert npc % NB == 0

    core = edge_dst // npc
    blk = (edge_dst - core * npc) // NB
    dstl = (edge_dst - core * npc) - blk * NB

    buckets = [[None] * nblk for _ in range(ncores)]
    order = np.lexsort((blk, core))
    core_s, blk_s = core[order], blk[order]
    bounds = np.searchsorted(core_s * nblk + blk_s, np.arange(ncores * nblk + 1))
    for k in range(ncores):
        for b in range(nblk):
            i0, i1 = bounds[k * nblk + b], bounds[k * nblk + b + 1]
            buckets[k][b] = order[i0:i1]

    TBs = []
    for b in range(nblk):
        mx = max(len(buckets[k][b]) for k in range(ncores))
        TBs.append(max(1, -(-mx // P)))

    per_core = []
    for k in range(ncores):
        srcs, dls, ws = [], [], []
        for b in range(nblk):
            ids = buckets[k][b]
            pad = TBs[b] * P - len(ids)
            srcs.append(np.pad(edge_src[ids], (0, pad)))
            dls.append(np.pad(dstl[ids], (0, pad)))
            ws.append(np.pad(edge_weight[ids], (0, pad)))
        src = np.concatenate(srcs).astype(np.int64)
        dl = np.concatenate(dls).astype(np.float32)
        w = np.concatenate(ws).astype(np.float32)

        def idx16(a):
            # dma_gather layout: idx j at [j%16, j//16], replicated on 8
            # 16-partition groups -> [128, n/16]
            a16 = a.astype(np.int16).reshape(-1, 16).T
            return np.tile(a16, (8, 1)).copy()

        per_core.append(dict(
            src16=idx16(src),
            src_glob=src,
            dst_glob=np.concatenate(
                [np.pad(edge_dst[buckets[k][b]],
                        (0, TBs[b] * P - len(buckets[k][b])))
                 for b in range(nblk)]).astype(np.int64),
            dstl=dl.reshape(-1, P).T.copy(),     # [128, TT]
            wts=w.reshape(-1, P).T.copy(),       # [128, TT]
        ))
    return TBs, per_core


# ------------------------------------------------------------------ builders

def _new_nc():
    return bacc.Bacc(None, target_bir_lowering=False)


def _iota_row(nc, pool):
    ji = pool.tile([P, NB], mybir.dt.int32)
    nc.gpsimd.iota(ji[:], pattern=[[1, NB]], base=0, channel_multiplier=0)
    j = pool.tile([P, NB], mybir.dt.float32)
    nc.vector.tensor_copy(j[:], ji[:])
    return j


def _build_l1(TBs, cfg, reps=1):
    """deg -> dinv, output [NB, nblk] f32 (node r of block b at [r, b])."""
    nc = _new_nc()
    nblk = len(TBs)
    TT = sum(TBs)

    dstl_in = nc.dram_tensor("dstl", [P, TT], mybir.dt.float32, kind="ExternalInput")
    wts_in = nc.dram_tensor("wts", [P, TT], mybir.dt.float32, kind="ExternalInput")
    dinv_out = nc.dram_tensor("dinvc", [NB, nblk], mybir.dt.float32,
                              kind="ExternalOutput")

    with TileContext(nc) as tc:
        with (
            tc.tile_pool(name="sbA", bufs=1) as sbA,
            tc.tile_pool(name="sbS", bufs=4) as sbS,
            tc.tile_pool(name="ps", bufs=2, space="PSUM") as ps,
        ):
          for _rep in range(reps):
            J = _iota_row(nc, sbA)
            dstl = sbA.tile([P, TT], mybir.dt.float32)
            wts = sbA.tile([P, TT], mybir.dt.float32)
            nc.sync.dma_start(out=dstl[:], in_=dstl_in[:])
            nc.sync.dma_start(out=wts[:], in_=wts_in[:])
            ones_f = sbA.tile([P, 1], mybir.dt.float32)
            nc.vector.memset(ones_f[:], 1.0)
            ones = sbA.tile([P, 1], mybir.dt.bfloat16)
            nc.vector.tensor_copy(ones[:], ones_f[:])

            degc = sbA.tile([NB, nblk], mybir.dt.float32)
            toff = 0
            for b in range(nblk):
                Tb = TBs[b]
                pd = ps.tile([NB, 1], mybir.dt.float32, space="PSUM", tag="deg")
                for t in range(Tb):
                    col = toff + t
                    S0w = sbS.tile([P, NB], mybir.dt.bfloat16, tag="S0w")
                    nc.vector.tensor_scalar(
                        out=S0w[:], in0=J[:],
                        scalar1=dstl[:, col:col + 1], scalar2=wts[:, col:col + 1],
                        op0=mybir.AluOpType.is_equal, op1=mybir.AluOpType.mult)
                    nc.tensor.matmul(out=pd[:], lhsT=S0w[:], rhs=ones[:],
                                     start=(t == 0), stop=(t == Tb - 1))
                nc.vector.tensor_copy(degc[:, b:b + 1], pd[:])
                toff += Tb
            # dinv = sqrt(1/max(deg, deg==0 ? 1)) * (deg > 0)
            m = sbA.tile([NB, nblk], mybir.dt.float32)
            nc.vector.tensor_scalar(out=m[:], in0=degc[:], scalar1=0.0,
                                    scalar2=None, op0=mybir.AluOpType.is_gt)
            le = sbA.tile([NB, nblk], mybir.dt.float32)
            nc.vector.tensor_scalar(out=le[:], in0=degc[:], scalar1=0.0,
                                    scalar2=None, op0=mybir.AluOpType.is_le)
            safe = sbA.tile([NB, nblk], mybir.dt.float32)
            nc.vector.tensor_tensor(out=safe[:], in0=degc[:], in1=le[:],
                                    op=mybir.AluOpType.add)
            rec = sbA.tile([NB, nblk], mybir.dt.float32)
            nc.vector.reciprocal(rec[:], safe[:])
            sq = sbA.tile([NB, nblk], mybir.dt.float32)
            nc.scalar.sqrt(sq[:], rec[:])
            dv = sbA.tile([NB, nblk], mybir.dt.float32)
            nc.vector.tensor_tensor(out=dv[:], in0=sq[:], in1=m[:],
                                    op=mybir.AluOpType.mult)
            nc.sync.dma_start(out=dinv_out[:], in_=dv[:])
    nc.finalize()
    return nc


def _build_l2(TBs, cfg, reps=1, dbg=False):
    nc = _new_nc()
    nblk = len(TBs)
    TT = sum(TBs)
    N, HID, OUT, B = cfg["N"], cfg["HID"], cfg["OUT"], cfg["B"]
    npc = N // cfg["CORES"]
    HCH = HID // P          # 8
    OCH = OUT // P          # 13
    NCW = 500
    NCH = npc // NCW        # 5
    KW = [(0, P), (P, P), (2 * P, F_TEXT - 2 * P)]   # contraction chunks of 300

    nf_in = nc.dram_tensor("nf16", [N, FPAD], mybir.dt.bfloat16, kind="ExternalInput")
    src_in = nc.dram_tensor("src16", [P, TT * 8], mybir.dt.int16, kind="ExternalInput")
    dstl_in = nc.dram_tensor("dstl", [P, TT], mybir.dt.float32, kind="ExternalInput")
    wts_in = nc.dram_tensor("wts", [P, TT], mybir.dt.float32, kind="ExternalInput")
    dsrc_in = nc.dram_tensor("dsrc", [P, TT], mybir.dt.float32, kind="ExternalInput")
    ddst_in = nc.dram_tensor("ddst", [P, TT], mybir.dt.float32, kind="ExternalInput")
    w1_in = nc.dram_tensor("w1b", [2 * P + KW[2][1], HID], mybir.dt.bfloat16,
                           kind="ExternalInput")
    w2t_in = nc.dram_tensor("w2t", [OUT, HID], mybir.dt.bfloat16, kind="ExternalInput")
    b1_in = nc.dram_tensor("b1c", [P, HCH], mybir.dt.float32, kind="ExternalInput")
    img_in = nc.dram_tensor("img", [B, OUT], mybir.dt.float32, kind="ExternalInput")
    b2_in = nc.dram_tensor("b2r", [B, OUT], mybir.dt.float32, kind="ExternalInput")

    qtt_out = nc.dram_tensor("qtt", [B, npc], mybir.dt.float32, kind="ExternalOutput")
    c_out = nc.dram_tensor("cvec", [B, 1], mybir.dt.float32, kind="ExternalOutput")
    if dbg:
        agg_outs = [nc.dram_tensor(f"dbg_agg{i}", [P, npc], mybir.dt.float32,
                                   kind="ExternalOutput") for i in range(3)]
        x1_out = nc.dram_tensor("dbg_x1", [P, HCH * npc], mybir.dt.float32,
                                kind="ExternalOutput")
        pt_out = nc.dram_tensor("dbg_pt", [P, HCH * B], mybir.dt.float32,
                                kind="ExternalOutput")

    with TileContext(nc) as tc:
        with (
            tc.tile_pool(name="sbA", bufs=1) as sbA,
            tc.tile_pool(name="sbG", bufs=3) as sbG,
            tc.tile_pool(name="sbS", bufs=4) as sbS,
            tc.tile_pool(name="sbW", bufs=2) as sbW,
            tc.tile_pool(name="ps1", bufs=1, space="PSUM") as ps1,
            tc.tile_pool(name="ps2", bufs=2, space="PSUM") as ps2,
            tc.tile_pool(name="ps3", bufs=1, space="PSUM") as ps3,
        ):
          for _rep in range(reps):
            J = _iota_row(nc, sbA)
            ident = sbA.tile([P, P], mybir.dt.float32)
            make_identity(nc, ident[:])

            src16 = sbA.tile([P, TT * 8], mybir.dt.int16)
            dstl = sbA.tile([P, TT], mybir.dt.float32)
            wts = sbA.tile([P, TT], mybir.dt.float32)
            dsrc = sbA.tile([P, TT], mybir.dt.float32)
            ddst = sbA.tile([P, TT], mybir.dt.float32)
            for t, src in ((src16, src_in), (dstl, dstl_in), (wts, wts_in),
                           (dsrc, dsrc_in), (ddst, ddst_in)):
                nc.sync.dma_start(out=t[:], in_=src[:])
            nrm = sbA.tile([P, TT], mybir.dt.float32)
            nc.vector.tensor_tensor(out=nrm[:], in0=wts[:], in1=dsrc[:],
                                    op=mybir.AluOpType.mult)
            nc.vector.tensor_tensor(out=nrm[:], in0=nrm[:], in1=ddst[:],
                                    op=mybir.AluOpType.mult)

            # ---------- phase A: imgT, PT = W2 @ imgT, cvec ----------------
            imgs = sbA.tile([B, OUT], mybir.dt.float32)
            nc.sync.dma_start(out=imgs[:], in_=img_in[:])
            imgT = sbA.tile([P, OCH * B], mybir.dt.bfloat16)
            for o in range(OCH):
                tps = ps2.tile([P, B], mybir.dt.float32, space="PSUM", tag="tr")
                nc.tensor.transpose(tps[:], imgs[:, o * P:(o + 1) * P],
                                    ident[:B, :B])
                nc.vector.tensor_copy(imgT[:, o * B:(o + 1) * B], tps[:])
            w2sb = sbA.tile([P, OCH, HID], mybir.dt.bfloat16)
            nc.sync.dma_start(
                out=w2sb[:],
                in_=bass.AP(w2t_in, 0, [[HID, P], [P * HID, OCH], [1, HID]]))
            pt_ps = ps3.tile([P, HCH * B], mybir.dt.float32, space="PSUM", tag="pt")
            for h in range(HCH):
                for o in range(OCH):
                    nc.tensor.matmul(
                        out=pt_ps[:, h * B:(h + 1) * B],
                        lhsT=w2sb[:, o, h * P:(h + 1) * P],
                        rhs=imgT[:, o * B:(o + 1) * B],
                        start=(o == 0), stop=(o == OCH - 1))
            PT = sbA.tile([P, HCH * B], mybir.dt.bfloat16)
            nc.vector.tensor_copy(PT[:], pt_ps[:])

            b2r = sbA.tile([B, OUT], mybir.dt.float32)
            nc.sync.dma_start(out=b2r[:], in_=b2_in[:])
            nc.vector.tensor_tensor(out=b2r[:], in0=imgs[:], in1=b2r[:],
                                    op=mybir.AluOpType.mult)
            c_sb = sbA.tile([B, 1], mybir.dt.float32)
            nc.vector.tensor_reduce(out=c_sb[:], in_=b2r[:],
                                    axis=mybir.AxisListType.X,
                                    op=mybir.AluOpType.add)
            nc.sync.dma_start(out=c_out[:], in_=c_sb[:])

            # ---------- phase B: gathers + layer-1 aggregation -------------
            agg = [sbA.tile([P, npc], mybir.dt.bfloat16, tag="agg0", name="agg0"),
                   sbA.tile([P, npc], mybir.dt.bfloat16, tag="agg1", name="agg1"),
                   sbA.tile([KW[2][1], npc], mybir.dt.bfloat16, tag="agg2",
                            name="agg2")]
            toff = 0
            for b in range(nblk):
                Tb = TBs[b]
                ni = Tb * P
                mg = sbG.tile([P, Tb, FPAD], mybir.dt.bfloat16, tag="mg")
                nc.gpsimd.dma_gather(
                    out_ap=mg[:], in_ap=nf_in[:],
                    idxs_ap=src16[:, toff * 8:(toff + Tb) * 8],
                    num_idxs=ni, num_idxs_reg=ni, elem_size=FPAD,
                    single_packet=False)
                pa = [ps1.tile([P, NB], mybir.dt.float32, space="PSUM", tag="pa0",
                               name="pa0"),
                      ps1.tile([P, NB], mybir.dt.float32, space="PSUM", tag="pa1",
                               name="pa1"),
                      ps1.tile([KW[2][1], NB], mybir.dt.float32, space="PSUM",
                               tag="pa2", name="pa2")]
                for t in range(Tb):
                    col = toff + t
                    S1 = sbS.tile([P, NB], mybir.dt.bfloat16, tag="S1")
                    nc.vector.tensor_scalar(
                        out=S1[:], in0=J[:],
                        scalar1=dstl[:, col:col + 1], scalar2=nrm[:, col:col + 1],
                        op0=mybir.AluOpType.is_equal, op1=mybir.AluOpType.mult)
                    for fc, (k0, kw) in enumerate(KW):
                        nc.tensor.matmul(
                            out=pa[fc][:, :],
                            lhsT=mg[:, t, k0:k0 + kw],
                            rhs=S1[:],
                            start=(t == 0), stop=(t == Tb - 1))
                for fc in range(3):
                    nc.vector.tensor_copy(agg[fc][:, b * NB:(b + 1) * NB], pa[fc][:])
                toff += Tb

            if dbg:
              with tc.tile_pool(name="sbDbgA", bufs=1) as sbDbg:
                for i in range(3):
                    af = sbDbg.tile([P, npc], mybir.dt.float32, tag=f"dbga{i}")
                    nc.vector.memset(af[:], 0.0)
                    nc.vector.tensor_copy(af[:agg[i].shape[0], :], agg[i][:])
                    nc.sync.dma_start(out=agg_outs[i][:], in_=af[:])

            # ---------- phase C: x1T = prelu(W1.T @ aggF + b1, 0.2) --------
            w1t = [sbA.tile([P, HID], mybir.dt.bfloat16, tag="w1k0", name="w1k0"),
                   sbA.tile([P, HID], mybir.dt.bfloat16, tag="w1k1", name="w1k1"),
                   sbA.tile([KW[2][1], HID], mybir.dt.bfloat16, tag="w1k2",
                            name="w1k2")]
            nc.sync.dma_start(out=w1t[0][:], in_=w1_in[0:P, :])
            nc.sync.dma_start(out=w1t[1][:], in_=w1_in[P:2 * P, :])
            nc.sync.dma_start(out=w1t[2][:], in_=w1_in[2 * P:2 * P + KW[2][1], :])
            b1c = sbA.tile([P, HCH], mybir.dt.float32)
            nc.sync.dma_start(out=b1c[:], in_=b1_in[:])

            x1T = [sbA.tile([P, npc], mybir.dt.bfloat16, tag=f"x1T{h}",
                            name=f"x1T{h}") for h in range(HCH)]
            for h in range(HCH):
                for nchi in range(NCH):
                    n0 = nchi * NCW
                    px = ps2.tile([P, NCW], mybir.dt.float32, space="PSUM", tag="xq")
                    for kc in range(3):
                        nc.tensor.matmul(
                            out=px[:],
                            lhsT=w1t[kc][:, h * P:(h + 1) * P],
                            rhs=agg[kc][:, n0:n0 + NCW],
                            start=(kc == 0), stop=(kc == 2))
                    nc.scalar.activation(
                        out=x1T[h][:, n0:n0 + NCW], in_=px[:],
                        func=mybir.ActivationFunctionType.Prelu,
                        bias=b1c[:, h:h + 1], scale=1.0, alpha=NEG)

            if dbg:
              with tc.tile_pool(name="sbDbgB", bufs=1) as sbDbg:
                ptf = sbDbg.tile([P, HCH * B], mybir.dt.float32, tag="dbgpt")
                nc.vector.tensor_copy(ptf[:], PT[:])
                nc.sync.dma_start(out=pt_out[:], in_=ptf[:])
                for h in range(HCH):
                    xf = sbDbg.tile([P, npc], mybir.dt.float32, tag="dbgx")
                    nc.vector.tensor_copy(xf[:], x1T[h][:])
                    nc.sync.dma_start(out=x1_out[:, h * npc:(h + 1) * npc],
                                      in_=xf[:])

            # ---------- phase D: QTT = PT.T @ x1T --------------------------
            for nchi in range(NCH):
                n0 = nchi * NCW
                pq = ps2.tile([B, NCW], mybir.dt.float32, space="PSUM", tag="xq")
                for h in range(HCH):
                    nc.tensor.matmul(
                        out=pq[:], lhsT=PT[:, h * B:(h + 1) * B],
                        rhs=x1T[h][:, n0:n0 + NCW],
                        start=(h == 0), stop=(h == HCH - 1))
                qsb = sbS.tile([B, NCW], mybir.dt.float32, tag="qsb")
                nc.vector.tensor_copy(qsb[:], pq[:])
                nc.sync.dma_start(out=qtt_out[:, n0:n0 + NCW], in_=qsb[:])
    nc.finalize()
    return nc


def _build_l3(TBs, cfg, reps=1):
    nc = _new_nc()
    nblk = len(TBs)
    TT = sum(TBs)
    N, B = cfg["N"], cfg["B"]
    npc = N // cfg["CORES"]

    qt_in = nc.dram_tensor("qt2", [N, P], mybir.dt.bfloat16, kind="ExternalInput")
    src_in = nc.dram_tensor("src16", [P, TT * 8], mybir.dt.int16, kind="ExternalInput")
    dstl_in = nc.dram_tensor("dstl", [P, TT], mybir.dt.float32, kind="ExternalInput")
    wts_in = nc.dram_tensor("wts", [P, TT], mybir.dt.float32, kind="ExternalInput")
    dsrc_in = nc.dram_tensor("dsrc", [P, TT], mybir.dt.float32, kind="ExternalInput")
    ddst_in = nc.dram_tensor("ddst", [P, TT], mybir.dt.float32, kind="ExternalInput")
    c_in = nc.dram_tensor("cvec", [B, 1], mybir.dt.float32, kind="ExternalInput")
    out_own = nc.dram_tensor("outp", [B, npc], mybir.dt.float32, kind="ExternalOutput")

    with TileContext(nc) as tc:
        with (
            tc.tile_pool(name="sbA", bufs=1) as sbA,
            tc.tile_pool(name="sbG", bufs=3) as sbG,
            tc.tile_pool(name="sbS", bufs=4) as sbS,
            tc.tile_pool(name="ps", bufs=2, space="PSUM") as ps,
        ):
          for _rep in range(reps):
            J = _iota_row(nc, sbA)
            src16 = sbA.tile([P, TT * 8], mybir.dt.int16)
            dstl = sbA.tile([P, TT], mybir.dt.float32)
            wts = sbA.tile([P, TT], mybir.dt.float32)
            dsrc = sbA.tile([P, TT], mybir.dt.float32)
            ddst = sbA.tile([P, TT], mybir.dt.float32)
            c_sb = sbA.tile([B, 1], mybir.dt.float32)
            for t, src in ((src16, src_in), (dstl, dstl_in), (wts, wts_in),
                           (dsrc, dsrc_in), (ddst, ddst_in), (c_sb, c_in)):
                nc.sync.dma_start(out=t[:], in_=src[:])
            nrm = sbA.tile([P, TT], mybir.dt.float32)
            nc.vector.tensor_tensor(out=nrm[:], in0=wts[:], in1=dsrc[:],
                                    op=mybir.AluOpType.mult)
            nc.vector.tensor_tensor(out=nrm[:], in0=nrm[:], in1=ddst[:],
                                    op=mybir.AluOpType.mult)

            toff = 0
            for b in range(nblk):
                Tb = TBs[b]
                ni = Tb * P
                qg = sbG.tile([P, Tb, P], mybir.dt.bfloat16, tag="qg")
                nc.gpsimd.dma_gather(
                    out_ap=qg[:], in_ap=qt_in[:],
                    idxs_ap=src16[:, toff * 8:(toff + Tb) * 8],
                    num_idxs=ni, num_idxs_reg=ni, elem_size=P, single_packet=False)
                po = ps.tile([B, NB], mybir.dt.float32, space="PSUM", tag="po")
                for t in range(Tb):
                    col = toff + t
                    S1 = sbS.tile([P, NB], mybir.dt.bfloat16, tag="S1")
                    nc.vector.tensor_scalar(
                        out=S1[:], in0=J[:],
                        scalar1=dstl[:, col:col + 1], scalar2=nrm[:, col:col + 1],
                        op0=mybir.AluOpType.is_equal, op1=mybir.AluOpType.mult)
                    nc.tensor.matmul(out=po[:], lhsT=qg[:, t, 0:B], rhs=S1[:],
                                     start=(t == 0), stop=(t == Tb - 1))
                osb = sbS.tile([B, NB], mybir.dt.float32, tag="osb")
                nc.vector.tensor_scalar(out=osb[:], in0=po[:],
                                        scalar1=c_sb[:, 0:1], scalar2=None,
                                        op0=mybir.AluOpType.add)
                nc.sync.dma_start(out=out_own[:, b * NB:(b + 1) * NB], in_=osb[:])
                toff += Tb
    nc.finalize()
    return nc


# ------------------------------------------------------------------- runner

def _run(name, nc, in_maps, cores):
    kw = {}
    if TRACE:
        kw = dict(trace=True)
    res = run_bass_kernel_spmd(nc, in_maps, core_ids=list(range(cores)), **kw)
    if res.exec_time_ns is not None:
        LAST_EXEC_NS[name] = res.exec_time_ns
    return res.results


def _kernel_impl(img_feat, node_features, edge_src, edge_dst, edge_weight,
                 W1, b1, W2, b2, cfg):
    ncores = cfg["CORES"]
    N, B, HID, OUT = cfg["N"], cfg["B"], cfg["HID"], cfg["OUT"]
    npc = N // ncores
    nblk = npc // NB

    TBs, per_core = _prep_edges(edge_src, edge_dst, edge_weight, cfg)
    key = (tuple(TBs), tuple(sorted(cfg.items())))
    if key not in _BUILD_CACHE:
        _BUILD_CACHE[key] = (_build_l1(TBs, cfg), _build_l2(TBs, cfg),
                             _build_l3(TBs, cfg))
    nc1, nc2, nc3 = _BUILD_CACHE[key]

    # ---- L1: dinv per core [NB, nblk]
    maps1 = [{"dstl": pc["dstl"], "wts": pc["wts"]} for pc in per_core]
    r1 = _run("l1", nc1, maps1, ncores)
    dinv_all = np.concatenate(
        [r1[k]["dinvc"].T.reshape(-1) for k in range(ncores)])   # [N] layout only

    # ---- L2
    nf16 = np.zeros((N, FPAD), ml_dtypes.bfloat16)
    nf16[:, :F_TEXT] = node_features.astype(ml_dtypes.bfloat16)
    w1b = np.zeros((2 * P + (F_TEXT - 2 * P), HID), ml_dtypes.bfloat16)
    w1b[:F_TEXT, :] = W1.astype(ml_dtypes.bfloat16)
    w2t = np.ascontiguousarray(W2.T).astype(ml_dtypes.bfloat16)
    b1c = np.ascontiguousarray(b1.reshape(HID // P, P).T).astype(np.float32)
    b2rep = np.ascontiguousarray(np.broadcast_to(b2, (B, OUT))).astype(np.float32)
    img32 = img_feat.astype(np.float32)

    def edge_cols(vals, pc):
        # [TT*128] -> [128, TT] (column t = edges t*128..t*128+127)
        return np.ascontiguousarray(vals.reshape(-1, P).T)

    maps2 = []
    for pc in per_core:
        dsrc = edge_cols(dinv_all[pc["src_glob"]].astype(np.float32), pc)
        ddst = edge_cols(dinv_all[pc["dst_glob"]].astype(np.float32), pc)
        pc["dsrc"], pc["ddst"] = dsrc, ddst
        maps2.append(dict(nf16=nf16, src16=pc["src16"], dstl=pc["dstl"],
                          wts=pc["wts"], dsrc=dsrc, ddst=ddst, w1b=w1b,
                          w2t=w2t, b1c=b1c, img=img32, b2r=b2rep))
    r2 = _run("l2", nc2, maps2, ncores)
    cvec = r2[0]["cvec"]
    # qt table [N, 128] bf16 (64 data cols duplicated to fill 256B rows)
    qt16 = np.concatenate(
        [r2[k]["qtt"].T for k in range(ncores)]).astype(ml_dtypes.bfloat16)
    qt2 = np.concatenate([qt16, qt16], axis=1)

    # ---- L3
    maps3 = [dict(qt2=qt2, src16=pc["src16"], dstl=pc["dstl"], wts=pc["wts"],
                  dsrc=pc["dsrc"], ddst=pc["ddst"], cvec=cvec)
             for pc in per_core]
    r3 = _run("l3", nc3, maps3, ncores)
    out = np.concatenate([r3[k]["outp"] for k in range(ncores)], axis=1)

    global DBG
    DBG = dict(dinv_all=dinv_all, r2=r2, qt2=qt2, cvec=cvec,
               per_core=per_core, TBs=TBs)

    global LAST_BUILD, LAST_MAPS, LAUNCH_NAMES, BUILDERS
    LAST_BUILD = (nc1, nc2, nc3)
    LAST_MAPS = {"l1": maps1, "l2": maps2, "l3": maps3}
    LAUNCH_NAMES = ("l1", "l2", "l3")
    BUILDERS = {"l1": lambda reps=1: _build_l1(TBs, cfg, reps),
                "l2": lambda reps=1: _build_l2(TBs, cfg, reps),
                "l3": lambda reps=1: _build_l3(TBs, cfg, reps)}
    return out.astype(np.float32)


def kernel(img_feat, node_features, edge_src, edge_dst, edge_weight,
           W1, b1, W2, b2):
    return _kernel_impl(np.asarray(img_feat), np.asarray(node_features),
                        np.asarray(edge_src), np.asarray(edge_dst),
                        np.asarray(edge_weight), np.asarray(W1),
                        np.asarray(b1), np.asarray(W2), np.asarray(b2),
                        CFG_FULL)
